# revision 39
# baseline (speedup 1.0000x reference)
"""Trainium2 Bass kernel for MultiCNNTransformerBlock_vis.

Data-parallel over batch: 8 batch elements -> 8 NeuronCores, one each.
Per-core computation (B_local=1, D=256, NH=4, DH=64, H=W=32, L=S=1024):
  q = conv3x3(x, Wq); k/v = multi-scale convs (1/3/5/7) of source
  vis = softmax(q k^T / 8) per head           -> [4, 1024, 1024]
  linear attention (elu+1 features) -> merge -> LN -> FF -> LN -> +x

Layout strategy: feature-major [channels(part), pixels(free)] everywhere;
convs are PE matmuls over shifted padded images (k and v convs of equal
kernel size packed into one 128-wide stationary operand); channel-dim
reductions (layernorm, linear-attn denominator) via ones-vector matmuls;
softmax tiles are [l(part), s(free)] with the denominator taken from the
activation accumulator.
"""

import sys
from contextlib import ExitStack

import numpy as np

try:
    import concourse  # noqa: F401
except ImportError:
    sys.path.append("/opt/trn_rl_repo")

import concourse.bass as bass  # noqa: E402
import concourse.tile as tile  # noqa: E402
from concourse import bacc, mybir  # noqa: E402
from concourse.bass_utils import run_bass_kernel_spmd  # noqa: E402

F32 = mybir.dt.float32
MR = mybir.dt.float32r  # reduced-precision matmul operand format (TF32-class)
AF = mybir.ActivationFunctionType
OP = mybir.AluOpType

D = 256
NH = 4
DH = 64
HH = 32
L = HH * HH  # 1024
NCH = 2  # channel chunks of 128
TEMP = DH ** -0.5
EPS_LN = 1e-5
EPS_ATT = 1e-6
KV_SIZES = (1, 3, 5, 7)

_CACHE = {}
DEBUG_TAPS = []  # names: kf0,kf1,v0,v1,ksum,z_all,kv_sb,msgT0,msgT1,mg0,mg1,ff10


def _emit(ctx: ExitStack, tc: "tile.TileContext", io: dict):
    nc = tc.nc

    # ---------------- pools ----------------
    consts = ctx.enter_context(tc.tile_pool(name="consts", bufs=1))
    wconv = ctx.enter_context(tc.tile_pool(name="wconv", bufs=6))
    feats = ctx.enter_context(tc.tile_pool(name="feats", bufs=12 if DEBUG_TAPS else 7))
    big = ctx.enter_context(tc.tile_pool(name="big", bufs=4))
    tmp4k = ctx.enter_context(tc.tile_pool(name="tmp4k", bufs=1))
    expp = ctx.enter_context(tc.tile_pool(name="expp", bufs=7))
    accp = ctx.enter_context(tc.tile_pool(name="accp", bufs=2))
    smalls = ctx.enter_context(tc.tile_pool(name="smalls", bufs=1))
    blk = ctx.enter_context(tc.tile_pool(name="blk", bufs=2))
    psum = ctx.enter_context(tc.tile_pool(name="psum", bufs=2, space="PSUM"))
    psum_t = ctx.enter_context(tc.tile_pool(name="psum_t", bufs=2, space="PSUM"))
    psum_kv = ctx.enter_context(tc.tile_pool(name="psum_kv", bufs=1, space="PSUM"))
    psum_st = ctx.enter_context(tc.tile_pool(name="psum_st", bufs=2, space="PSUM"))

    # ---------------- constants / small weights (allocated now; their DMAs
    # are emitted after the conv weights so they don't delay the first conv)
    ident = consts.tile([128, 128], MR, tag="ident")
    mean_col = consts.tile([128, 1], MR, tag="mean_col")
    ones_row = consts.tile([1, 128], MR, tag="ones_row")
    neg_row = consts.tile([1, 128], MR, tag="neg_row")
    sel = [consts.tile([128, 128], MR, tag=f"sel{c}", name=f"sel{c}")
           for c in range(NCH)]
    gb = [consts.tile([128, 4], F32, tag=f"gb{c}", name=f"gb{c}")
          for c in range(NCH)]  # [128, 4]: ln1g ln1b ln2g ln2b
    w1_sb = consts.tile([128, 4, 512], MR, tag="w1")
    w2_sb = consts.tile([128, 4, 256], MR, tag="w2")
    wm_sb = consts.tile([128, 2, 256], MR, tag="wm")
    eps_ln = consts.tile([1, 1], F32, tag="eps_ln")

    def load_attn_consts():
        nc.sync.dma_start(out=ident, in_=io["ident"])
        for c in range(NCH):
            nc.sync.dma_start(out=sel[c], in_=io["sel"][c])

    def load_ff_consts():
        nc.sync.dma_start(out=mean_col, in_=io["mean_col"])
        nc.sync.dma_start(out=ones_row, in_=io["ones_row"])
        nc.sync.dma_start(out=neg_row, in_=io["neg_row"])
        for c in range(NCH):
            nc.sync.dma_start(out=gb[c], in_=io["gb"][c])
        nc.sync.dma_start(out=w1_sb, in_=io["w1"])
        nc.sync.dma_start(out=w2_sb, in_=io["w2"])
        nc.sync.dma_start(out=wm_sb, in_=io["wm"])
        nc.vector.memset(eps_ln, EPS_LN)

    # ---------------- padded input images ----------------
    # x padded to 34x34 (pad 1), source padded to 38x38 (pad 3)
    xpad = []
    for c in range(NCH):
        t = consts.tile([128, 34, 34], MR, tag=f"xpad{c}", name=f"xpad{c}")
        nc.sync.dma_start(out=t, in_=io["xp"][c])
        xpad.append(t)
    spad = []
    for c in range(NCH):
        t = big.tile([128, 38, 38], MR, tag="big", name=f"spad{c}")
        nc.sync.dma_start(out=t, in_=io["sp"][c])
        spad.append(t)

    # feature-major activation buffers [128, 1024] x 2 chunks
    q_sb = [feats.tile([128, L], MR, tag="fm", name=f"q{c}") for c in range(NCH)]
    k_sb = [feats.tile([128, L], MR, tag="fm", name=f"k{c}") for c in range(NCH)]
    v_sb = [feats.tile([128, L], MR, tag="fm", name=f"v{c}") for c in range(NCH)]

    # shared linear-attention state
    # head h's Z row lives at partition 32*h (engine start partitions are
    # 32-aligned); untouched rows are zeroed so the selector matmul stays clean
    z_all = smalls.tile([128, L], MR, tag="zall")
    nc.vector.memset(z_all[:].bitcast(F32), 0.0)
    ksum = smalls.tile([128, 2], MR, tag="ksum")
    kv_sb = smalls.tile([128, 2, 64], MR, tag="kvsb")
    msgT = [feats.tile([128, L], MR, tag="fm", name=f"msgT{c}") for c in range(NCH)]
    # one PSUM bank per head PAIR for KV accumulation: a matmul with
    # start=True clears its whole bank, so interleaved accumulation groups
    # must not share one.
    kv_ps = [psum_kv.tile([128, 128], F32, tag=f"kvp{p}", name=f"kvps{p}")
             for p in range(2)]

    def kv_conv(si):
        # weight tiles are split into groups of <=25 offsets so every conv
        # weight tile fits one uniform 12.5KB pool slot (kv7's 49 offsets
        # would otherwise double the slot size for everyone)
        ksz = KV_SIZES[si]
        noff = ksz * ksz
        pad = ksz // 2
        ogroups = [(g, min(25, noff - g)) for g in range(0, noff, 25)]
        wt = {}
        for c in range(NCH):
            for g0, glen in ogroups:
                t = wconv.tile([128, glen, 128], MR, tag="convw",
                               name=f"wkv{ksz}_{c}_{g0}")
                nc.sync.dma_start(out=t, in_=io[f"wkv{ksz}"][c][:, g0:g0 + glen, :])
                wt[(c, g0)] = t
        cch, roff = si // 2, (si % 2) * 64
        for n in range(NCH):
            ps = psum.tile([128, 512], F32, tag="mm", name="psmm")
            nmm = NCH * noff
            i = 0
            for c in range(NCH):
                for g0, glen in ogroups:
                    for oo in range(glen):
                        o = g0 + oo
                        dy, dx = o // ksz, o % ksz
                        r0 = 16 * n + 3 - pad + dy
                        c0 = 3 - pad + dx
                        rhs = spad[c][:, r0:r0 + 16, c0:c0 + 32]
                        nc.tensor.matmul(ps, wt[(c, g0)][:, oo, :], rhs,
                                         start=(i == 0), stop=(i == nmm - 1))
                        i += 1
            nc.vector.tensor_copy(k_sb[cch][roff:roff + 64, 512 * n:512 * (n + 1)],
                                  ps[0:64])
            nc.vector.tensor_copy(v_sb[cch][roff:roff + 64, 512 * n:512 * (n + 1)],
                                  ps[64:128])

    def q_conv():
        wq = {}
        for c in range(NCH):
            for oc in range(NCH):
                t = wconv.tile([128, 9, 128], MR, tag="convw", name=f"wq{c}{oc}")
                nc.sync.dma_start(out=t,
                                  in_=io["wq"][c][:, :, oc * 128:(oc + 1) * 128])
                wq[(c, oc)] = t
        for oc in range(NCH):
            for n in range(NCH):
                ps = psum.tile([128, 512], F32, tag="mm", name="psmm")
                i = 0
                for c in range(NCH):
                    for o in range(9):
                        dy, dx = o // 3, o % 3
                        rhs = xpad[c][:, 16 * n + dy:16 * n + dy + 16, dx:dx + 32]
                        nc.tensor.matmul(ps, wq[(c, oc)][:, o, :],
                                         rhs, start=(i == 0), stop=(i == 17))
                        i += 1
                nc.vector.tensor_copy(q_sb[oc][:, 512 * n:512 * (n + 1)], ps)

    def qk_softmax(cch):
        # both heads of chunk cch; softmax denominators batched in groups of
        # 4 l-chunks per head (one reduce + one reciprocal per group).
        for lg in range(2):           # l-chunk group
            for hh in range(2):
                h = 2 * cch + hh
                ro = hh * 64
                acc = accp.tile([128, 4, 2], F32, tag="acc", name=f"ac{cch}{hh}{lg}")
                ets = []
                for li in range(4):
                    lc = lg * 4 + li
                    et = expp.tile([128, L], F32, tag="exp", name=f"e{cch}{hh}{lc}")
                    ets.append(et)
                    for n in range(NCH):
                        ps = psum.tile([128, 512], F32, tag="mm", name="psmm")
                        nc.tensor.matmul(
                            ps,
                            q_sb[cch][ro:ro + 64, lc * 128:(lc + 1) * 128],
                            k_sb[cch][ro:ro + 64, 512 * n:512 * (n + 1)],
                            start=True, stop=True,
                        )
                        nc.scalar.activation(et[:, 512 * n:512 * (n + 1)], ps,
                                             AF.Exp, scale=TEMP,
                                             accum_out=acc[:, li, n:n + 1])
                den = accp.tile([128, 4], F32, tag="den", name=f"dn{cch}{hh}{lg}")
                nc.vector.tensor_reduce(den, acc[:], axis=mybir.AxisListType.X,
                                        op=OP.add)
                nc.vector.reciprocal(den[:], den[:])
                for li in range(4):
                    lc = lg * 4 + li
                    nc.vector.tensor_scalar_mul(ets[li][:], ets[li][:],
                                                den[:, li:li + 1])
                    nc.gpsimd.dma_start(
                        out=io["vis"][h, lc * 128:(lc + 1) * 128, :],
                        in_=ets[li][:])

    def linattn_chunk(c):
        # elu feature maps in place over q/k chunk c (after QK matmuls read
        # the raw values), then Z denominators, Z fold, and the KV/msg path
        # for heads 2c and 2c+1 (which live entirely in channel chunk c).
        # elu(x)+1 = exp(min(x,0)) + (x - min(x,0)) with a single temp
        for src_t in (q_sb, k_sb):
            t1 = tmp4k.tile([128, L], F32, tag="t4k", name="elu1")
            nc.gpsimd.tensor_scalar_min(t1[:], src_t[c][:], 0.0)
            nc.vector.tensor_sub(src_t[c][:], src_t[c][:], t1[:])
            nc.scalar.activation(t1[:], t1[:], AF.Exp)
            nc.vector.tensor_add(src_t[c][:], src_t[c][:], t1[:])
        qf, kf = q_sb, k_sb  # now hold elu(x)+1 for chunk c
        nc.vector.tensor_reduce(ksum[:, c:c + 1], kf[c][:],
                                axis=mybir.AxisListType.X, op=OP.add)
        # Z = 1 / (Qf . ksum + eps) per head; ksum is the 1-column
        # stationary operand, so no elementwise pre-multiply is needed
        for hh in range(2):
            h = 2 * c + hh
            ro = hh * 64
            zrow = 32 * h
            for n in range(NCH):
                dps = psum_st.tile([1, 512], F32, tag="stat", name="psden")
                nc.tensor.matmul(dps, ksum[ro:ro + 64, c:c + 1],
                                 qf[c][ro:ro + 64, 512 * n:512 * (n + 1)],
                                 start=True, stop=True)
                nc.vector.tensor_scalar_add(
                    z_all[zrow:zrow + 1, 512 * n:512 * (n + 1)], dps, EPS_ATT)
            nc.vector.reciprocal(z_all[zrow:zrow + 1, :], z_all[zrow:zrow + 1, :])
        # fold Z into Qf
        for n in range(NCH):
            zps = psum.tile([128, 512], F32, tag="mm", name="pszb")
            nc.tensor.matmul(zps, sel[c], z_all[:, 512 * n:512 * (n + 1)],
                             start=True, stop=True)
            nc.vector.tensor_mul(qf[c][:, 512 * n:512 * (n + 1)],
                                 qf[c][:, 512 * n:512 * (n + 1)], zps)
        # KV for this chunk's head pair: transpose 128-blocks, accumulate.
        # The [128,128] pair matmul computes cross-head blocks for free; only
        # the diagonal blocks are used.
        for b in range(8):
            kfb = blk.tile([128, 128], MR, tag="kfb", name=f"kfb{c}{b}")
            vb = blk.tile([128, 128], MR, tag="vb", name=f"vb{c}{b}")
            tp = psum_t.tile([128, 128], MR, tag="tp", name="pstp")
            nc.tensor.transpose(tp, kf[c][:, b * 128:(b + 1) * 128], ident[:])
            nc.scalar.copy(kfb[:], tp)
            tp2 = psum_t.tile([128, 128], MR, tag="tp", name="pstp2")
            nc.tensor.transpose(tp2, v_sb[c][:, b * 128:(b + 1) * 128], ident[:])
            nc.scalar.copy(vb[:], tp2)
            nc.tensor.matmul(kv_ps[c], kfb[:], vb[:],
                             start=(b == 0), stop=(b == 7), skip_group_check=True)
        for hh in range(2):
            h = 2 * c + hh
            nc.scalar.copy(kv_sb[hh * 64:hh * 64 + 64, c, :],
                           kv_ps[c][hh * 64:hh * 64 + 64, hh * 64:hh * 64 + 64])
        # msg^T[v, l] = sum_d KV[d, v] * Qf_z[d, l]   (feature-major)
        for hh in range(2):
            ro = hh * 64
            for n in range(NCH):
                ps = psum.tile([128, 512], F32, tag="mm", name="psmm")
                nc.tensor.matmul(ps[0:64, :], kv_sb[ro:ro + 64, c, :],
                                 qf[c][ro:ro + 64, 512 * n:512 * (n + 1)],
                                 start=True, stop=True)
                nc.scalar.copy(msgT[c][ro:ro + 64, 512 * n:512 * (n + 1)],
                               ps[0:64, :])

    # ---- schedule: overlap softmax/linattn (ACT/DVE/DMA) with convs (PE) ----
    q_conv()
    kv_conv(2)
    kv_conv(3)
    qk_softmax(1)
    kv_conv(0)
    kv_conv(1)
    load_attn_consts()
    linattn_chunk(1)
    qk_softmax(0)
    load_ff_consts()
    linattn_chunk(0)

    # ---------------- merge (msg @ Wm) feature-major + LN1 ----------------
    merged = [feats.tile([128, L], MR, tag="fm", name=f"mg{c}") for c in range(NCH)]
    for n in range(NCH):
        for oc in range(NCH):
            ps = psum.tile([128, 512], F32, tag="mm", name="psmm")
            for dc in range(NCH):
                nc.tensor.matmul(ps, wm_sb[:, dc, oc * 128:(oc + 1) * 128],
                                 msgT[dc][:, 512 * n:512 * (n + 1)],
                                 start=(dc == 0), stop=(dc == 1))
            nc.scalar.copy(merged[oc][:, 512 * n:512 * (n + 1)], ps)

    _layernorm_featmajor(nc, psum, psum_st, big, smalls, merged,
                         mean_col, ones_row, neg_row, gb, 0, eps_ln)

    # ---------------- FF: relu(cat(x, msg) @ W1) @ W2 + LN2 ----------------
    ff1 = [big.tile([128, L], MR, tag="big", name=f"ff1_{m}") for m in range(4)]
    for n in range(NCH):
        for mc in range(4):
            ps = psum.tile([128, 512], F32, tag="mm", name="psmm")
            for fc in range(4):
                if fc < 2:
                    rhs = xpad[fc][:, 16 * n + 1:16 * n + 17, 1:33]
                else:
                    rhs = merged[fc - 2][:, 512 * n:512 * (n + 1)]
                nc.tensor.matmul(ps, w1_sb[:, fc, mc * 128:(mc + 1) * 128], rhs,
                                 start=(fc == 0), stop=(fc == 3))
            nc.scalar.activation(ff1[mc][:, 512 * n:512 * (n + 1)], ps, AF.Relu)
    ff2 = [feats.tile([128, L], MR, tag="fm", name=f"ff2_{c}") for c in range(NCH)]
    for n in range(NCH):
        for oc in range(NCH):
            ps = psum.tile([128, 512], F32, tag="mm", name="psmm")
            for mc in range(4):
                nc.tensor.matmul(ps, w2_sb[:, mc, oc * 128:(oc + 1) * 128],
                                 ff1[mc][:, 512 * n:512 * (n + 1)],
                                 start=(mc == 0), stop=(mc == 3))
            nc.scalar.copy(ff2[oc][:, 512 * n:512 * (n + 1)], ps)

    _layernorm_featmajor(nc, psum, psum_st, big, smalls, ff2,
                         mean_col, ones_row, neg_row, gb, 2, eps_ln,
                         residual=(xpad, big, io))

    # ---------------- debug taps ----------------
    tapmap = {"kf0": k_sb[0], "kf1": k_sb[1], "v0": v_sb[0], "v1": v_sb[1],
              "ksum": ksum, "z_all": z_all, "kv_sb": kv_sb,
              "msgT0": msgT[0], "msgT1": msgT[1],
              "mg0": merged[0], "mg1": merged[1], "ff10": ff1[0],
              "ff20": ff2[0], "ff21": ff2[1]}
    for tname in DEBUG_TAPS:
        nc.sync.dma_start(out=io["tap_" + tname], in_=tapmap[tname][:])


def _layernorm_featmajor(nc, psum, psum_st, big, smalls, chunks,
                         mean_col, ones_row, neg_row, gb, gb_off, eps_ln,
                         residual=None):
    """In-place layernorm over the channel (partition) dim of 2 chunk tiles.

    mean_col carries 1/D so the ones-matmuls produce means directly;
    neg_row carries -1 so the t-broadcast needs no separate negation.
    Processed per 512-pixel slice so the serial stat chain pipelines.
    With residual=(xpad, pool, io), adds x and stores to io["out"] per slice.
    """
    sq = [big.tile([128, L], MR, tag="big", name=f"lnsq{c}_{gb_off}")
          for c in range(NCH)]
    for c in range(NCH):
        nc.scalar.square(sq[c][:], chunks[c][:])
    for n in range(NCH):
        s1 = smalls.tile([1, 512], MR, tag="lns1", name=f"lns1_{gb_off}{n}")
        s2 = smalls.tile([1, 512], MR, tag="lns2", name=f"lns2_{gb_off}{n}")
        p1 = psum_st.tile([1, 512], F32, tag="stat", name="pslns1")
        p2 = psum_st.tile([1, 512], F32, tag="stat", name="pslns2")
        for c in range(NCH):
            nc.tensor.matmul(p1, mean_col, chunks[c][:, 512 * n:512 * (n + 1)],
                             start=(c == 0), stop=(c == 1))
        for c in range(NCH):
            nc.tensor.matmul(p2, mean_col, sq[c][:, 512 * n:512 * (n + 1)],
                             start=(c == 0), stop=(c == 1))
        nc.vector.tensor_copy(s1[:], p1)  # m
        nc.vector.tensor_copy(s2[:], p2)  # E[x^2]
        # inv = 1/sqrt(E[x^2] - m^2 + eps);  t = -m*inv (negation via neg_row)
        msq = big.tile([1, 512], F32, tag="lnmsq", name=f"lnmsq_{gb_off}{n}",
                       bufs=1)
        nc.vector.tensor_mul(msq[:], s1[:], s1[:])
        nc.vector.tensor_sub(s2[:], s2[:], msq[:])
        nc.scalar.activation(s2[:], s2[:], AF.Sqrt, bias=eps_ln[:])
        nc.vector.reciprocal(s2[:], s2[:])
        nc.vector.tensor_mul(s1[:], s1[:], s2[:])  # m * inv
        bps_s = psum.tile([128, 512], F32, tag="mm", name="psbcs")
        nc.tensor.matmul(bps_s, ones_row, s2[:], start=True, stop=True)
        bps_t = psum.tile([128, 512], F32, tag="mm", name="psbct")
        nc.tensor.matmul(bps_t, neg_row, s1[:], start=True, stop=True)
        for c in range(NCH):
            sl = chunks[c][:, 512 * n:512 * (n + 1)]
            nc.vector.tensor_mul(sl, sl, bps_s)
            nc.vector.tensor_add(sl, sl, bps_t)
            nc.scalar.activation(sl, sl, AF.Identity,
                                 bias=gb[c][:, gb_off + 1:gb_off + 2],
                                 scale=gb[c][:, gb_off:gb_off + 1])
            if residual is not None:
                xpad, pool, io = residual
                ot = pool.tile([128, 512], F32, tag="otile", name=f"ot{c}{n}",
                               bufs=4)
                nc.gpsimd.tensor_add(
                    ot[:].rearrange("p (a b) -> p a b", a=16),
                    xpad[c][:, 16 * n + 1:16 * n + 17, 1:33],
                    sl.rearrange("p (a b) -> p a b", a=16))
                nc.gpsimd.dma_start(
                    out=io["out"][c * 128:(c + 1) * 128,
                                  512 * n:512 * (n + 1)],
                    in_=ot[:])


def _build_program(passes=1):
    nc = bacc.Bacc("TRN2", target_bir_lowering=False, debug=False, num_devices=8)
    io = {}

    def din(name, shape, dt=F32):
        io[name] = nc.dram_tensor(name, list(shape), dt, kind="ExternalInput").ap()

    din("xp", (NCH, 128, 34, 34), MR)
    din("sp", (NCH, 128, 38, 38), MR)
    din("wq", (NCH, 128, 9, D), MR)
    for ksz in KV_SIZES:
        din(f"wkv{ksz}", (NCH, 128, ksz * ksz, 128), MR)
    din("wm", (128, 2, D), MR)
    din("w1", (128, 4, 512), MR)
    din("w2", (128, 4, D), MR)
    din("gb", (NCH, 128, 4))
    din("ident", (128, 128), MR)
    din("mean_col", (128, 1), MR)
    din("ones_row", (1, 128), MR)
    din("neg_row", (1, 128), MR)
    din("sel", (NCH, 128, 128), MR)
    tap_shapes = {"kf0": (128, L), "kf1": (128, L), "v0": (128, L),
                  "v1": (128, L), "ksum": (128, 2), "z_all": (128, L),
                  "kv_sb": (128, 2, 64), "msgT0": (128, L), "msgT1": (128, L),
                  "mg0": (128, L), "mg1": (128, L), "ff10": (128, L),
                  "ff20": (128, L), "ff21": (128, L)}
    for tname in DEBUG_TAPS:
        io["tap_" + tname] = nc.dram_tensor(
            "tap_" + tname, list(tap_shapes[tname]), F32, kind="ExternalOutput").ap()
    io["out"] = nc.dram_tensor("out", [D, L], F32, kind="ExternalOutput").ap()
    io["vis"] = nc.dram_tensor("vis", [NH, L, L], F32, kind="ExternalOutput").ap()

    with tile.TileContext(nc) as tc:
        # float32r is a full 4-byte storage format; reductions into it
        # only lose the same mantissa bits the PE would drop anyway
        with nc.allow_low_precision(reason="float32r matmul operands"):
            for _ in range(passes):
                with ExitStack() as ctx:
                    _emit(ctx, tc, io)
    nc.compile()
    return nc


def _host_weights(inputs):
    """Pre-transform weights on host into matmul-ready layouts (shared by cores)."""
    f = np.float32
    out = {}
    wq = np.asarray(inputs["Wq"], f)  # [256, 256, 3, 3] (O, I, kh, kw)
    # lhsT layout [ic, oc] per offset -> [icchunk, ic128, off, oc]
    wq_t = wq.transpose(2, 3, 1, 0).reshape(9, D, D)  # [off, ic, oc]
    out["wq"] = np.ascontiguousarray(
        wq_t.reshape(9, NCH, 128, D).transpose(1, 2, 0, 3))
    for ksz in KV_SIZES:
        wk = np.asarray(inputs[f"Wk{ksz}"], f)  # [64, 256, k, k]
        wv = np.asarray(inputs[f"Wv{ksz}"], f)
        wkv = np.concatenate([wk, wv], axis=0)  # [128, 256, k, k]
        t = wkv.transpose(2, 3, 1, 0).reshape(ksz * ksz, D, 128)
        out[f"wkv{ksz}"] = np.ascontiguousarray(
            t.reshape(ksz * ksz, NCH, 128, 128).transpose(1, 2, 0, 3))
    wm = np.asarray(inputs["Wm"], f)  # [256, 256] (in, out)
    out["wm"] = np.ascontiguousarray(wm.reshape(2, 128, D).transpose(1, 0, 2))
    out["w1"] = np.ascontiguousarray(
        np.asarray(inputs["W1"], f).reshape(4, 128, 512).transpose(1, 0, 2))
    out["w2"] = np.ascontiguousarray(
        np.asarray(inputs["W2"], f).reshape(4, 128, D).transpose(1, 0, 2))
    gbm = np.stack([np.asarray(inputs["ln1_g"], f), np.asarray(inputs["ln1_b"], f),
                    np.asarray(inputs["ln2_g"], f), np.asarray(inputs["ln2_b"], f)],
                   axis=1)  # [256, 4]
    out["gb"] = np.ascontiguousarray(gbm.reshape(NCH, 128, 4))
    out["ident"] = np.eye(128, dtype=f)
    out["ones_col"] = np.ones((128, 1), f)
    out["mean_col"] = np.full((128, 1), 1.0 / D, f)
    out["ones_row"] = np.ones((1, 128), f)
    out["neg_row"] = np.full((1, 128), -1.0, f)
    sel = np.zeros((NCH, 128, 128), f)
    for c in range(NCH):
        for hh in range(2):
            h = 2 * c + hh
            sel[c, 32 * h, hh * 64:(hh + 1) * 64] = 1.0
    out["sel"] = sel
    return out


def kernel(**inputs):
    if "nc" not in _CACHE:
        _CACHE["nc"] = _build_program()
    nc = _CACHE["nc"]

    shared = _host_weights(inputs)
    x = np.asarray(inputs["x"], np.float32)  # [8, 256, 32, 32]
    src = np.asarray(inputs["source"], np.float32)
    B = x.shape[0]
    xp = np.zeros((B, NCH, 128, 34, 34), np.float32)
    xp[:, :, :, 1:33, 1:33] = x.reshape(B, NCH, 128, HH, HH)
    sp = np.zeros((B, NCH, 128, 38, 38), np.float32)
    sp[:, :, :, 3:35, 3:35] = src.reshape(B, NCH, 128, HH, HH)
    in_maps = []
    for b in range(B):
        m = dict(shared)
        m["xp"] = xp[b]
        m["sp"] = sp[b]
        in_maps.append(m)
    res = run_bass_kernel_spmd(nc, in_maps, list(range(B))).results
    out = np.stack([r["out"] for r in res]).reshape(B, D, HH, HH)
    vis = np.stack([r["vis"] for r in res]).reshape(B, NH, HH, HH, HH, HH)
    return out, vis


# revision 41
# speedup vs baseline: 13.2559x; 13.2559x over previous
"""Trainium2 Bass kernel for MultiCNNTransformerBlock_vis.

Data-parallel over batch: 8 batch elements -> 8 NeuronCores, one each.
Per-core computation (B_local=1, D=256, NH=4, DH=64, H=W=32, L=S=1024):
  q = conv3x3(x, Wq); k/v = multi-scale convs (1/3/5/7) of source
  vis = softmax(q k^T / 8) per head           -> [4, 1024, 1024]
  linear attention (elu+1 features) -> merge -> LN -> FF -> LN -> +x

Layout strategy: feature-major [channels(part), pixels(free)] everywhere;
convs are PE matmuls over shifted padded images (k and v convs of equal
kernel size packed into one 128-wide stationary operand); channel-dim
reductions (layernorm, linear-attn denominator) via ones-vector matmuls;
softmax tiles are [l(part), s(free)] with the denominator taken from the
activation accumulator.
"""

import sys
from contextlib import ExitStack

import numpy as np

try:
    import concourse  # noqa: F401
except ImportError:
    sys.path.append("/opt/trn_rl_repo")

import concourse.bass as bass  # noqa: E402
import concourse.tile as tile  # noqa: E402
from concourse import bacc, mybir  # noqa: E402
from concourse.bass_utils import run_bass_kernel_spmd  # noqa: E402

F32 = mybir.dt.float32
MR = mybir.dt.float32r  # reduced-precision matmul operand format (TF32-class)
AF = mybir.ActivationFunctionType
OP = mybir.AluOpType

D = 256
NH = 4
DH = 64
HH = 32
L = HH * HH  # 1024
NCH = 2  # channel chunks of 128
TEMP = DH ** -0.5
EPS_LN = 1e-5
EPS_ATT = 1e-6
KV_SIZES = (1, 3, 5, 7)

_CACHE = {}
DEBUG_TAPS = []  # names: kf0,kf1,v0,v1,ksum,z_all,kv_sb,msgT0,msgT1,mg0,mg1,ff10


def _emit(ctx: ExitStack, tc: "tile.TileContext", io: dict):
    nc = tc.nc

    # ---------------- pools ----------------
    consts = ctx.enter_context(tc.tile_pool(name="consts", bufs=1))
    wconv = ctx.enter_context(tc.tile_pool(name="wconv", bufs=6))
    feats = ctx.enter_context(tc.tile_pool(name="feats", bufs=12 if DEBUG_TAPS else 7))
    big = ctx.enter_context(tc.tile_pool(name="big", bufs=4))
    tmp4k = ctx.enter_context(tc.tile_pool(name="tmp4k", bufs=1))
    expp = ctx.enter_context(tc.tile_pool(name="expp", bufs=7))
    accp = ctx.enter_context(tc.tile_pool(name="accp", bufs=2))
    smalls = ctx.enter_context(tc.tile_pool(name="smalls", bufs=1))
    blk = ctx.enter_context(tc.tile_pool(name="blk", bufs=2))
    psum = ctx.enter_context(tc.tile_pool(name="psum", bufs=3, space="PSUM"))
    psum_t = ctx.enter_context(tc.tile_pool(name="psum_t", bufs=2, space="PSUM"))
    psum_kv = ctx.enter_context(tc.tile_pool(name="psum_kv", bufs=1, space="PSUM"))
    psum_st = ctx.enter_context(tc.tile_pool(name="psum_st", bufs=2, space="PSUM"))

    # ---------------- constants / small weights (allocated now; their DMAs
    # are emitted after the conv weights so they don't delay the first conv)
    ident = consts.tile([128, 128], MR, tag="ident")
    mean_col = consts.tile([128, 1], MR, tag="mean_col")
    ones_row = consts.tile([1, 128], MR, tag="ones_row")
    neg_row = consts.tile([1, 128], MR, tag="neg_row")
    sel = [consts.tile([128, 128], MR, tag=f"sel{c}", name=f"sel{c}")
           for c in range(NCH)]
    gb = [consts.tile([128, 4], F32, tag=f"gb{c}", name=f"gb{c}")
          for c in range(NCH)]  # [128, 4]: ln1g ln1b ln2g ln2b
    w1_sb = consts.tile([128, 4, 512], MR, tag="w1")
    w2_sb = consts.tile([128, 4, 256], MR, tag="w2")
    wm_sb = consts.tile([128, 2, 256], MR, tag="wm")
    eps_ln = consts.tile([1, 1], F32, tag="eps_ln")

    def load_attn_consts():
        nc.sync.dma_start(out=ident, in_=io["ident"])
        for c in range(NCH):
            nc.sync.dma_start(out=sel[c], in_=io["sel"][c])

    def load_ff_consts():
        nc.sync.dma_start(out=mean_col, in_=io["mean_col"])
        nc.sync.dma_start(out=ones_row, in_=io["ones_row"])
        nc.sync.dma_start(out=neg_row, in_=io["neg_row"])
        for c in range(NCH):
            nc.sync.dma_start(out=gb[c], in_=io["gb"][c])
        nc.sync.dma_start(out=w1_sb, in_=io["w1"])
        nc.sync.dma_start(out=w2_sb, in_=io["w2"])
        nc.sync.dma_start(out=wm_sb, in_=io["wm"])
        nc.vector.memset(eps_ln, EPS_LN)

    # ---------------- padded input images ----------------
    # x padded to 34x34 (pad 1), source padded to 38x38 (pad 3)
    xpad = []
    for c in range(NCH):
        t = consts.tile([128, 34, 34], MR, tag=f"xpad{c}", name=f"xpad{c}")
        nc.sync.dma_start(out=t, in_=io["xp"][c])
        xpad.append(t)
    spad = [big.tile([128, 38, 38], MR, tag="big", name=f"spad{c}")
            for c in range(NCH)]

    def load_spad():
        for c in range(NCH):
            nc.sync.dma_start(out=spad[c], in_=io["sp"][c])

    # feature-major activation buffers [128, 1024] x 2 chunks
    q_sb = [feats.tile([128, L], MR, tag="fm", name=f"q{c}") for c in range(NCH)]
    k_sb = [feats.tile([128, L], MR, tag="fm", name=f"k{c}") for c in range(NCH)]
    v_sb = [feats.tile([128, L], MR, tag="fm", name=f"v{c}") for c in range(NCH)]

    # shared linear-attention state
    # head h's Z row lives at partition 32*h (engine start partitions are
    # 32-aligned); untouched rows are zeroed so the selector matmul stays clean
    z_all = smalls.tile([128, L], MR, tag="zall")
    nc.vector.memset(z_all[:].bitcast(F32), 0.0)
    ksum = smalls.tile([128, 2], MR, tag="ksum")
    kv_sb = smalls.tile([128, 2, 64], MR, tag="kvsb")
    msgT = [feats.tile([128, L], MR, tag="fm", name=f"msgT{c}") for c in range(NCH)]

    def kv_conv(si):
        # weight tiles are split into groups of <=25 offsets so every conv
        # weight tile fits one uniform 12.5KB pool slot (kv7's 49 offsets
        # would otherwise double the slot size for everyone)
        ksz = KV_SIZES[si]
        noff = ksz * ksz
        pad = ksz // 2
        ogroups = [(g, min(25, noff - g)) for g in range(0, noff, 25)]
        wt = {}
        for c in range(NCH):
            for g0, glen in ogroups:
                t = wconv.tile([128, glen, 128], MR, tag="convw",
                               name=f"wkv{ksz}_{c}_{g0}")
                nc.sync.dma_start(out=t, in_=io[f"wkv{ksz}"][c][:, g0:g0 + glen, :])
                wt[(c, g0)] = t
        cch, roff = si // 2, (si % 2) * 64
        for n in range(NCH):
            ps = psum.tile([128, 512], F32, tag="mm", name="psmm")
            nmm = NCH * noff
            i = 0
            for c in range(NCH):
                for g0, glen in ogroups:
                    for oo in range(glen):
                        o = g0 + oo
                        dy, dx = o // ksz, o % ksz
                        r0 = 16 * n + 3 - pad + dy
                        c0 = 3 - pad + dx
                        rhs = spad[c][:, r0:r0 + 16, c0:c0 + 32]
                        nc.tensor.matmul(ps, wt[(c, g0)][:, oo, :], rhs,
                                         start=(i == 0), stop=(i == nmm - 1))
                        i += 1
            nc.vector.tensor_copy(k_sb[cch][roff:roff + 64, 512 * n:512 * (n + 1)],
                                  ps[0:64])
            nc.vector.tensor_copy(v_sb[cch][roff:roff + 64, 512 * n:512 * (n + 1)],
                                  ps[64:128])

    def q_conv():
        wq = {}
        for c in range(NCH):
            for oc in range(NCH):
                t = wconv.tile([128, 9, 128], MR, tag="convw", name=f"wq{c}{oc}")
                nc.sync.dma_start(out=t,
                                  in_=io["wq"][c][:, :, oc * 128:(oc + 1) * 128])
                wq[(c, oc)] = t
        for oc in range(NCH):
            for n in range(NCH):
                ps = psum.tile([128, 512], F32, tag="mm", name="psmm")
                i = 0
                for c in range(NCH):
                    for o in range(9):
                        dy, dx = o // 3, o % 3
                        rhs = xpad[c][:, 16 * n + dy:16 * n + dy + 16, dx:dx + 32]
                        nc.tensor.matmul(ps, wq[(c, oc)][:, o, :],
                                         rhs, start=(i == 0), stop=(i == 17))
                        i += 1
                nc.vector.tensor_copy(q_sb[oc][:, 512 * n:512 * (n + 1)], ps)

    def qk_softmax(cch):
        # both heads of chunk cch; softmax denominators batched in groups of
        # 4 l-chunks per head (one reduce + one reciprocal per group).
        for lg in range(2):           # l-chunk group
            for hh in range(2):
                h = 2 * cch + hh
                ro = hh * 64
                acc = accp.tile([128, 4, 2], F32, tag="acc", name=f"ac{cch}{hh}{lg}")
                ets = []
                for li in range(4):
                    lc = lg * 4 + li
                    et = expp.tile([128, L], F32, tag="exp", name=f"e{cch}{hh}{lc}")
                    ets.append(et)
                    for n in range(NCH):
                        ps = psum.tile([128, 512], F32, tag="mm", name="psmm")
                        nc.tensor.matmul(
                            ps,
                            q_sb[cch][ro:ro + 64, lc * 128:(lc + 1) * 128],
                            k_sb[cch][ro:ro + 64, 512 * n:512 * (n + 1)],
                            start=True, stop=True,
                        )
                        nc.scalar.activation(et[:, 512 * n:512 * (n + 1)], ps,
                                             AF.Exp, scale=TEMP,
                                             accum_out=acc[:, li, n:n + 1])
                den = accp.tile([128, 4], F32, tag="den", name=f"dn{cch}{hh}{lg}")
                nc.vector.tensor_reduce(den, acc[:], axis=mybir.AxisListType.X,
                                        op=OP.add)
                nc.vector.reciprocal(den[:], den[:])
                for li in range(4):
                    lc = lg * 4 + li
                    nc.vector.tensor_scalar_mul(ets[li][:], ets[li][:],
                                                den[:, li:li + 1])
                    nc.gpsimd.dma_start(
                        out=io["vis"][h, lc * 128:(lc + 1) * 128, :],
                        in_=ets[li][:])

    def linattn_chunk(c):
        # elu feature maps in place over q/k chunk c (after QK matmuls read
        # the raw values), then Z denominators, Z fold, and the KV/msg path
        # for heads 2c and 2c+1 (which live entirely in channel chunk c).
        # elu(x)+1 = exp(min(x,0)) + (x - min(x,0)) with a single temp
        for src_t in (q_sb, k_sb):
            t1 = tmp4k.tile([128, L], F32, tag="t4k", name="elu1")
            nc.gpsimd.tensor_scalar_min(t1[:], src_t[c][:], 0.0)
            nc.vector.tensor_sub(src_t[c][:], src_t[c][:], t1[:])
            nc.scalar.activation(t1[:], t1[:], AF.Exp)
            nc.vector.tensor_add(src_t[c][:], src_t[c][:], t1[:])
        qf, kf = q_sb, k_sb  # now hold elu(x)+1 for chunk c
        nc.vector.tensor_reduce(ksum[:, c:c + 1], kf[c][:],
                                axis=mybir.AxisListType.X, op=OP.add)
        # Z = 1 / (Qf . ksum + eps) per head; ksum is the 1-column
        # stationary operand, so no elementwise pre-multiply is needed
        for hh in range(2):
            h = 2 * c + hh
            ro = hh * 64
            zrow = 32 * h
            for n in range(NCH):
                dps = psum_st.tile([1, 512], F32, tag="stat", name="psden")
                nc.tensor.matmul(dps, ksum[ro:ro + 64, c:c + 1],
                                 qf[c][ro:ro + 64, 512 * n:512 * (n + 1)],
                                 start=True, stop=True)
                nc.vector.tensor_scalar_add(
                    z_all[zrow:zrow + 1, 512 * n:512 * (n + 1)], dps, EPS_ATT)
            nc.vector.reciprocal(z_all[zrow:zrow + 1, :], z_all[zrow:zrow + 1, :])
        # fold Z into Qf
        for n in range(NCH):
            zps = psum.tile([128, 512], F32, tag="mm", name="pszb")
            nc.tensor.matmul(zps, sel[c], z_all[:, 512 * n:512 * (n + 1)],
                             start=True, stop=True)
            nc.vector.tensor_mul(qf[c][:, 512 * n:512 * (n + 1)],
                                 qf[c][:, 512 * n:512 * (n + 1)], zps)
        # KV for this chunk's head pair: transpose 128-blocks, accumulate.
        # The [128,128] pair matmul computes cross-head blocks for free; only
        # the diagonal blocks are used.  One shared bank, reused per chunk
        # (the two chunks' accumulation windows are disjoint in time).
        kv_ps_c = psum_kv.tile([128, 128], F32, tag="kvp", name=f"kvps{c}")
        for b in range(8):
            kfb = blk.tile([128, 128], MR, tag="kfb", name=f"kfb{c}{b}")
            vb = blk.tile([128, 128], MR, tag="vb", name=f"vb{c}{b}")
            tp = psum_t.tile([128, 128], MR, tag="tp", name="pstp")
            nc.tensor.transpose(tp, kf[c][:, b * 128:(b + 1) * 128], ident[:])
            nc.scalar.copy(kfb[:], tp)
            tp2 = psum_t.tile([128, 128], MR, tag="tp", name="pstp2")
            nc.tensor.transpose(tp2, v_sb[c][:, b * 128:(b + 1) * 128], ident[:])
            nc.scalar.copy(vb[:], tp2)
            nc.tensor.matmul(kv_ps_c, kfb[:], vb[:],
                             start=(b == 0), stop=(b == 7), skip_group_check=True)
        for hh in range(2):
            h = 2 * c + hh
            nc.scalar.copy(kv_sb[hh * 64:hh * 64 + 64, c, :],
                           kv_ps_c[hh * 64:hh * 64 + 64, hh * 64:hh * 64 + 64])
        # msg^T[v, l] = sum_d KV[d, v] * Qf_z[d, l]   (feature-major)
        for hh in range(2):
            ro = hh * 64
            for n in range(NCH):
                ps = psum.tile([128, 512], F32, tag="mm", name="psmm")
                nc.tensor.matmul(ps[0:64, :], kv_sb[ro:ro + 64, c, :],
                                 qf[c][ro:ro + 64, 512 * n:512 * (n + 1)],
                                 start=True, stop=True)
                nc.scalar.copy(msgT[c][ro:ro + 64, 512 * n:512 * (n + 1)],
                               ps[0:64, :])

    # ---- schedule: overlap softmax/linattn (ACT/DVE/DMA) with convs (PE) ----
    q_conv()
    load_spad()
    kv_conv(2)
    kv_conv(3)
    qk_softmax(1)
    kv_conv(0)
    kv_conv(1)
    load_attn_consts()
    linattn_chunk(1)
    qk_softmax(0)
    load_ff_consts()
    linattn_chunk(0)

    # ---------------- merge (msg @ Wm) feature-major + LN1 ----------------
    merged = [feats.tile([128, L], MR, tag="fm", name=f"mg{c}") for c in range(NCH)]
    for n in range(NCH):
        for oc in range(NCH):
            ps = psum.tile([128, 512], F32, tag="mm", name="psmm")
            for dc in range(NCH):
                nc.tensor.matmul(ps, wm_sb[:, dc, oc * 128:(oc + 1) * 128],
                                 msgT[dc][:, 512 * n:512 * (n + 1)],
                                 start=(dc == 0), stop=(dc == 1))
            nc.scalar.copy(merged[oc][:, 512 * n:512 * (n + 1)], ps)

    _layernorm_featmajor(nc, psum, psum_st, big, smalls, merged,
                         mean_col, ones_row, neg_row, gb, 0, eps_ln)

    # ---------------- FF: relu(cat(x, msg) @ W1) @ W2 + LN2 ----------------
    ff1 = [big.tile([128, L], MR, tag="big", name=f"ff1_{m}") for m in range(4)]
    for n in range(NCH):
        for mc in range(4):
            ps = psum.tile([128, 512], F32, tag="mm", name="psmm")
            for fc in range(4):
                if fc < 2:
                    rhs = xpad[fc][:, 16 * n + 1:16 * n + 17, 1:33]
                else:
                    rhs = merged[fc - 2][:, 512 * n:512 * (n + 1)]
                nc.tensor.matmul(ps, w1_sb[:, fc, mc * 128:(mc + 1) * 128], rhs,
                                 start=(fc == 0), stop=(fc == 3))
            nc.scalar.activation(ff1[mc][:, 512 * n:512 * (n + 1)], ps, AF.Relu)
    ff2 = [feats.tile([128, L], MR, tag="fm", name=f"ff2_{c}") for c in range(NCH)]
    for n in range(NCH):
        for oc in range(NCH):
            ps = psum.tile([128, 512], F32, tag="mm", name="psmm")
            for mc in range(4):
                nc.tensor.matmul(ps, w2_sb[:, mc, oc * 128:(oc + 1) * 128],
                                 ff1[mc][:, 512 * n:512 * (n + 1)],
                                 start=(mc == 0), stop=(mc == 3))
            nc.scalar.copy(ff2[oc][:, 512 * n:512 * (n + 1)], ps)

    _layernorm_featmajor(nc, psum, psum_st, big, smalls, ff2,
                         mean_col, ones_row, neg_row, gb, 2, eps_ln,
                         residual=(xpad, big, io))

    # ---------------- debug taps ----------------
    tapmap = {"kf0": k_sb[0], "kf1": k_sb[1], "v0": v_sb[0], "v1": v_sb[1],
              "ksum": ksum, "z_all": z_all, "kv_sb": kv_sb,
              "msgT0": msgT[0], "msgT1": msgT[1],
              "mg0": merged[0], "mg1": merged[1], "ff10": ff1[0],
              "ff20": ff2[0], "ff21": ff2[1]}
    for tname in DEBUG_TAPS:
        nc.sync.dma_start(out=io["tap_" + tname], in_=tapmap[tname][:])


def _layernorm_featmajor(nc, psum, psum_st, big, smalls, chunks,
                         mean_col, ones_row, neg_row, gb, gb_off, eps_ln,
                         residual=None):
    """In-place layernorm over the channel (partition) dim of 2 chunk tiles.

    mean_col carries 1/D so the ones-matmuls produce means directly;
    neg_row carries -1 so the t-broadcast needs no separate negation.
    Processed per 512-pixel slice so the serial stat chain pipelines.
    With residual=(xpad, pool, io), adds x and stores to io["out"] per slice.
    """
    sq = [big.tile([128, L], MR, tag="big", name=f"lnsq{c}_{gb_off}")
          for c in range(NCH)]
    for c in range(NCH):
        nc.scalar.square(sq[c][:], chunks[c][:])
    for n in range(NCH):
        s1 = smalls.tile([1, 512], MR, tag="lns1", name=f"lns1_{gb_off}{n}")
        s2 = smalls.tile([1, 512], MR, tag="lns2", name=f"lns2_{gb_off}{n}")
        p1 = psum_st.tile([1, 512], F32, tag="stat", name="pslns1")
        p2 = psum_st.tile([1, 512], F32, tag="stat", name="pslns2")
        for c in range(NCH):
            nc.tensor.matmul(p1, mean_col, chunks[c][:, 512 * n:512 * (n + 1)],
                             start=(c == 0), stop=(c == 1))
        for c in range(NCH):
            nc.tensor.matmul(p2, mean_col, sq[c][:, 512 * n:512 * (n + 1)],
                             start=(c == 0), stop=(c == 1))
        nc.vector.tensor_copy(s1[:], p1)  # m
        nc.vector.tensor_copy(s2[:], p2)  # E[x^2]
        # inv = 1/sqrt(E[x^2] - m^2 + eps);  t = -m*inv (negation via neg_row)
        msq = big.tile([1, 512], F32, tag="lnmsq", name=f"lnmsq_{gb_off}{n}",
                       bufs=1)
        nc.vector.tensor_mul(msq[:], s1[:], s1[:])
        nc.vector.tensor_sub(s2[:], s2[:], msq[:])
        nc.scalar.activation(s2[:], s2[:], AF.Sqrt, bias=eps_ln[:])
        nc.vector.reciprocal(s2[:], s2[:])
        nc.vector.tensor_mul(s1[:], s1[:], s2[:])  # m * inv
        bps_s = psum.tile([128, 512], F32, tag="mm", name="psbcs")
        nc.tensor.matmul(bps_s, ones_row, s2[:], start=True, stop=True)
        bps_t = psum.tile([128, 512], F32, tag="mm", name="psbct")
        nc.tensor.matmul(bps_t, neg_row, s1[:], start=True, stop=True)
        for c in range(NCH):
            sl = chunks[c][:, 512 * n:512 * (n + 1)]
            nc.vector.tensor_mul(sl, sl, bps_s)
            nc.vector.tensor_add(sl, sl, bps_t)
            nc.scalar.activation(sl, sl, AF.Identity,
                                 bias=gb[c][:, gb_off + 1:gb_off + 2],
                                 scale=gb[c][:, gb_off:gb_off + 1])
            if residual is not None:
                xpad, pool, io = residual
                ot = pool.tile([128, 512], F32, tag="otile", name=f"ot{c}{n}",
                               bufs=4)
                nc.gpsimd.tensor_add(
                    ot[:].rearrange("p (a b) -> p a b", a=16),
                    xpad[c][:, 16 * n + 1:16 * n + 17, 1:33],
                    sl.rearrange("p (a b) -> p a b", a=16))
                nc.gpsimd.dma_start(
                    out=io["out"][c * 128:(c + 1) * 128,
                                  512 * n:512 * (n + 1)],
                    in_=ot[:])


def _build_program(passes=1):
    nc = bacc.Bacc("TRN2", target_bir_lowering=False, debug=False, num_devices=8)
    io = {}

    def din(name, shape, dt=F32):
        io[name] = nc.dram_tensor(name, list(shape), dt, kind="ExternalInput").ap()

    din("xp", (NCH, 128, 34, 34), MR)
    din("sp", (NCH, 128, 38, 38), MR)
    din("wq", (NCH, 128, 9, D), MR)
    for ksz in KV_SIZES:
        din(f"wkv{ksz}", (NCH, 128, ksz * ksz, 128), MR)
    din("wm", (128, 2, D), MR)
    din("w1", (128, 4, 512), MR)
    din("w2", (128, 4, D), MR)
    din("gb", (NCH, 128, 4))
    din("ident", (128, 128), MR)
    din("mean_col", (128, 1), MR)
    din("ones_row", (1, 128), MR)
    din("neg_row", (1, 128), MR)
    din("sel", (NCH, 128, 128), MR)
    tap_shapes = {"kf0": (128, L), "kf1": (128, L), "v0": (128, L),
                  "v1": (128, L), "ksum": (128, 2), "z_all": (128, L),
                  "kv_sb": (128, 2, 64), "msgT0": (128, L), "msgT1": (128, L),
                  "mg0": (128, L), "mg1": (128, L), "ff10": (128, L),
                  "ff20": (128, L), "ff21": (128, L)}
    for tname in DEBUG_TAPS:
        io["tap_" + tname] = nc.dram_tensor(
            "tap_" + tname, list(tap_shapes[tname]), F32, kind="ExternalOutput").ap()
    io["out"] = nc.dram_tensor("out", [D, L], F32, kind="ExternalOutput").ap()
    io["vis"] = nc.dram_tensor("vis", [NH, L, L], F32, kind="ExternalOutput").ap()

    with tile.TileContext(nc) as tc:
        # float32r is a full 4-byte storage format; reductions into it
        # only lose the same mantissa bits the PE would drop anyway
        with nc.allow_low_precision(reason="float32r matmul operands"):
            for _ in range(passes):
                with ExitStack() as ctx:
                    _emit(ctx, tc, io)
    nc.compile()
    return nc


def _host_weights(inputs):
    """Pre-transform weights on host into matmul-ready layouts (shared by cores)."""
    f = np.float32
    out = {}
    wq = np.asarray(inputs["Wq"], f)  # [256, 256, 3, 3] (O, I, kh, kw)
    # lhsT layout [ic, oc] per offset -> [icchunk, ic128, off, oc]
    wq_t = wq.transpose(2, 3, 1, 0).reshape(9, D, D)  # [off, ic, oc]
    out["wq"] = np.ascontiguousarray(
        wq_t.reshape(9, NCH, 128, D).transpose(1, 2, 0, 3))
    for ksz in KV_SIZES:
        wk = np.asarray(inputs[f"Wk{ksz}"], f)  # [64, 256, k, k]
        wv = np.asarray(inputs[f"Wv{ksz}"], f)
        wkv = np.concatenate([wk, wv], axis=0)  # [128, 256, k, k]
        t = wkv.transpose(2, 3, 1, 0).reshape(ksz * ksz, D, 128)
        out[f"wkv{ksz}"] = np.ascontiguousarray(
            t.reshape(ksz * ksz, NCH, 128, 128).transpose(1, 2, 0, 3))
    wm = np.asarray(inputs["Wm"], f)  # [256, 256] (in, out)
    out["wm"] = np.ascontiguousarray(wm.reshape(2, 128, D).transpose(1, 0, 2))
    out["w1"] = np.ascontiguousarray(
        np.asarray(inputs["W1"], f).reshape(4, 128, 512).transpose(1, 0, 2))
    out["w2"] = np.ascontiguousarray(
        np.asarray(inputs["W2"], f).reshape(4, 128, D).transpose(1, 0, 2))
    gbm = np.stack([np.asarray(inputs["ln1_g"], f), np.asarray(inputs["ln1_b"], f),
                    np.asarray(inputs["ln2_g"], f), np.asarray(inputs["ln2_b"], f)],
                   axis=1)  # [256, 4]
    out["gb"] = np.ascontiguousarray(gbm.reshape(NCH, 128, 4))
    out["ident"] = np.eye(128, dtype=f)
    out["ones_col"] = np.ones((128, 1), f)
    out["mean_col"] = np.full((128, 1), 1.0 / D, f)
    out["ones_row"] = np.ones((1, 128), f)
    out["neg_row"] = np.full((1, 128), -1.0, f)
    sel = np.zeros((NCH, 128, 128), f)
    for c in range(NCH):
        for hh in range(2):
            h = 2 * c + hh
            sel[c, 32 * h, hh * 64:(hh + 1) * 64] = 1.0
    out["sel"] = sel
    return out


def kernel(**inputs):
    if "nc" not in _CACHE:
        _CACHE["nc"] = _build_program()
    nc = _CACHE["nc"]

    shared = _host_weights(inputs)
    x = np.asarray(inputs["x"], np.float32)  # [8, 256, 32, 32]
    src = np.asarray(inputs["source"], np.float32)
    B = x.shape[0]
    xp = np.zeros((B, NCH, 128, 34, 34), np.float32)
    xp[:, :, :, 1:33, 1:33] = x.reshape(B, NCH, 128, HH, HH)
    sp = np.zeros((B, NCH, 128, 38, 38), np.float32)
    sp[:, :, :, 3:35, 3:35] = src.reshape(B, NCH, 128, HH, HH)
    in_maps = []
    for b in range(B):
        m = dict(shared)
        m["xp"] = xp[b]
        m["sp"] = sp[b]
        in_maps.append(m)
    res = run_bass_kernel_spmd(nc, in_maps, list(range(B))).results
    out = np.stack([r["out"] for r in res]).reshape(B, D, HH, HH)
    vis = np.stack([r["vis"] for r in res]).reshape(B, NH, HH, HH, HH, HH)
    return out, vis
